# revision 14
# baseline (speedup 1.0000x reference)
"""Trainium2 Bass kernel for the MgSmmS linear-RNN model.

Math: the reference computes, per batch b,
    h_t = W_A h_{t-1} + (x[b,t] * v + c),   v = W_B[:,0],  c = b_A + b_B + W_bh
    out = W_C h_S + b_C + x[b,S-1] W_D[:,0] + (b_D + b_J + W_J @ 1)
Unrolling the linear recurrence:
    h_S = sum_{j=0}^{S-1} W_A^j (x[b, S-1-j] v + c)
W_A entries are U(-1/64, 1/64), spectral radius ~0.577, so W_A^j decays by
~0.577 per step; past j ~ 26 the terms are below fp32 resolution of the
leading terms.  With T = 28:
    out[b, :] = sum_{s<T} x[b, S-1-s] * (W_C W_A^s v) + W_C d + consts,
    d = sum_{s<T} W_A^s c
so the device work is a T-step Krylov chain z_{s+1} = W_A z_s on the
2-column block z_0 = [v | c], plus per-step projections W_C z_s, plus one
tiny (B x T+1) @ (T+1 x OUT) matmul.

Precision: fp32 matmuls measure ~430 ns per 128x128 tile on TRN2 (2-pass
weight load + 2 half-rate passes) while bf16 sustains ~30 ns.  So the chain
runs entirely in bf16: the first S0 steps (and projections) use a hi/lo
split (A ~ A_hi + A_lo, z ~ z_hi + z_lo, keeping A_hi*z_hi + A_hi*z_lo +
A_lo*z_hi, fp32 PSUM accumulation) giving ~1e-5 relative accuracy where the
terms are large; later steps are plain bf16, their absolute contribution
already down by 0.577^S0.

Distribution: W_A^T is column-sharded across the 8 cores (bf16 hi+lo slabs,
4 MB each, SBUF-resident).  Each chain step, core k computes 512 rows of
z_{s+1} and an AllGather (4 KB per rank) rebuilds the full z on every core.
Projections of the previous z run on the PE while the AllGather flies.  The
final assembly is computed redundantly on every core; the host reads core 0.

Written in raw bass (explicit per-engine programs + semaphores): every
instruction carries at most one sync wait, standalone wait_ge instructions
do the rest.  DVE same-engine RAW hazards are broken with explicit drains.

Layouts: the hidden index is stored partition-major, SBUF position (p, t)
holding hidden index j = p*NJT + t, so every DRAM<->SBUF transfer is
contiguous per partition.  The per-core output slab is ordered r = p*NIT+it
(psum partition-major); the W_A^T slab's column order bakes in that
permutation, and the AllGather concat plus the partition-major re-read make
the global z consistent again.  All permutations are host-side numpy.
"""

import contextlib

import numpy as np

import concourse.bass as bass
import concourse.mybir as mybir
from concourse.bass_utils import run_bass_kernel_spmd

T = 28            # truncated chain length
S0 = 12           # hi/lo-accurate steps (chain from z_j for j < S0, proj j <= S0)
H = 4096
G = 2048
OUT = 64
B = 64
S = 512
NCORES = 8
HSH = H // NCORES  # 512 rows of z computed per core
NJT = H // 128     # 32 contraction tiles
NIT = HSH // 128   # 4 output tiles per core
FP32 = mybir.dt.float32
BF16 = mybir.dt.bfloat16

LAST_RESULT = None  # BassKernelResults of the most recent run (for test.py)


def _build():
    nc = bass.Bass(target_bir_lowering=False, debug=False)

    # Per-core inputs (the W_A^T slabs differ per core, the rest replicated).
    at_hi = nc.declare_dram_parameter("at_hi", [128, NJT, HSH], BF16, isOutput=False)
    at_lo = nc.declare_dram_parameter("at_lo", [128, NJT, HSH], BF16, isOutput=False)
    wct_hi = nc.declare_dram_parameter("wct_hi", [128, NJT, OUT], BF16, isOutput=False)
    wct_lo = nc.declare_dram_parameter("wct_lo", [128, NJT, OUT], BF16, isOutput=False)
    # vecs = [v, b_A, b_B, W_bh] packed
    vecs = nc.declare_dram_parameter("vecs", [128, 4, NJT], FP32, isOutput=False)
    wj = nc.declare_dram_parameter("wj", [OUT, G], FP32, isOutput=False)
    # bvec columns = [b_C, b_D, b_J, W_D[:, 0]]
    bvec = nc.declare_dram_parameter("bvec", [OUT, 4], FP32, isOutput=False)
    xrt = nc.declare_dram_parameter("xrt", [T + 1, B], FP32, isOutput=False)
    out = nc.declare_dram_parameter("out", [B, OUT], FP32, isOutput=True)

    # Collective bounce buffers, one pair per chain step
    zslab = [nc.dram_tensor(f"zslab{s}", [HSH, 2], FP32) for s in range(T - 1)]
    zfull = [
        nc.dram_tensor(f"zfull{s}", [H, 2], FP32, addr_space="Shared")
        for s in range(T - 1)
    ]
    groups = [list(range(NCORES))]

    # --- SBUF ---
    at_hi_sb = nc.alloc_sbuf_tensor("at_hi_sb", [128, NJT, HSH], BF16).ap()
    at_lo_sb = nc.alloc_sbuf_tensor("at_lo_sb", [128, NJT, HSH], BF16).ap()
    wct_hi_sb = nc.alloc_sbuf_tensor("wct_hi_sb", [128, NJT, OUT], BF16).ap()
    wct_lo_sb = nc.alloc_sbuf_tensor("wct_lo_sb", [128, NJT, OUT], BF16).ap()
    vecs_sb = nc.alloc_sbuf_tensor("vecs_sb", [128, 4, NJT], FP32).ap()
    csum = nc.alloc_sbuf_tensor("csum", [128, NJT], FP32).ap()
    zbuf = [
        nc.alloc_sbuf_tensor(f"zbuf{i}", [128, NJT, 2], FP32).ap() for i in range(3)
    ]
    # bf16 [z_hi | z_lo] per ring slot, cols 0:2 hi, 2:4 lo
    zhl = [
        nc.alloc_sbuf_tensor(f"zhl{i}", [128, NJT, 4], BF16).ap() for i in range(3)
    ]
    zhi32 = nc.alloc_sbuf_tensor("zhi32", [128, NJT, 2], FP32).ap()
    ztmp = nc.alloc_sbuf_tensor("ztmp", [128, NJT, 2], FP32).ap()
    znext = [
        nc.alloc_sbuf_tensor(f"znext{i}", [128, NIT, 2], FP32).ap() for i in range(2)
    ]
    wj_sb = nc.alloc_sbuf_tensor("wj_sb", [OUT, G], FP32).ap()
    bvec_sb = nc.alloc_sbuf_tensor("bvec_sb", [OUT, 4], FP32).ap()
    ktilT = nc.alloc_sbuf_tensor("ktilT", [OUT, T + 1], FP32).ap()
    ktil = nc.alloc_sbuf_tensor("ktil", [T + 1, OUT], FP32).ap()
    xrt_sb = nc.alloc_sbuf_tensor("xrt_sb", [T + 1, B], FP32).ap()
    out_sb = nc.alloc_sbuf_tensor("out_sb", [B, OUT], FP32).ap()
    ident = nc.alloc_sbuf_tensor("ident", [OUT, OUT], FP32).ap()
    dsum = nc.alloc_sbuf_tensor("dsum", [OUT, 1], FP32).ap()
    wjsum = nc.alloc_sbuf_tensor("wjsum", [OUT, 1], FP32).ap()
    acc1 = nc.alloc_sbuf_tensor("acc1", [OUT, 1], FP32).ap()
    acc2 = nc.alloc_sbuf_tensor("acc2", [OUT, 1], FP32).ap()
    acc3 = nc.alloc_sbuf_tensor("acc3", [OUT, 1], FP32).ap()

    # --- PSUM (each tensor gets its own bank(s)) ---
    pszn = [nc.alloc_psum_tensor(f"pszn{i}", [128, 2], FP32).ap() for i in range(NIT)]
    proj = nc.alloc_psum_tensor("proj", [OUT, T, 2], FP32).ap()
    tp_ps = nc.alloc_psum_tensor("tp_ps", [T + 1, OUT], FP32).ap()
    out_ps = nc.alloc_psum_tensor("out_ps", [B, OUT], FP32).ap()

    with contextlib.ExitStack() as ctx:
        block = ctx.enter_context(nc.Block())
        s_athi = ctx.enter_context(nc.semaphore("s_athi"))
        s_atlo = ctx.enter_context(nc.semaphore("s_atlo"))
        s_wcthi = ctx.enter_context(nc.semaphore("s_wcthi"))
        s_wctlo = ctx.enter_context(nc.semaphore("s_wctlo"))
        s_vecs = ctx.enter_context(nc.semaphore("s_vecs"))
        s_wj = ctx.enter_context(nc.semaphore("s_wj"))
        s_bvec = ctx.enter_context(nc.semaphore("s_bvec"))
        s_xrt = ctx.enter_context(nc.semaphore("s_xrt"))
        s_zprep = ctx.enter_context(nc.semaphore("s_zprep"))
        s_zin = ctx.enter_context(nc.semaphore("s_zin"))
        s_mm = ctx.enter_context(nc.semaphore("s_mm"))
        s_cp = ctx.enter_context(nc.semaphore("s_cp"))
        s_slab = ctx.enter_context(nc.semaphore("s_slab"))
        s_cc = ctx.enter_context(nc.semaphore("s_cc"))
        s_proj = ctx.enter_context(nc.semaphore("s_proj"))
        s_ident = ctx.enter_context(nc.semaphore("s_ident"))
        s_ktilT = ctx.enter_context(nc.semaphore("s_ktilT"))
        s_tp = ctx.enter_context(nc.semaphore("s_tp"))
        s_ktil2 = ctx.enter_context(nc.semaphore("s_ktil2"))
        s_outmm = ctx.enter_context(nc.semaphore("s_outmm"))
        s_endout = ctx.enter_context(nc.semaphore("s_endout"))
        s_outdma = ctx.enter_context(nc.semaphore("s_outdma"))

        @block.sync
        def _(sync: bass.BassEngine):
            sync.dma_start(out=at_hi_sb, in_=at_hi[:]).then_inc(s_athi, 16)
            sync.dma_start(out=at_lo_sb, in_=at_lo[:]).then_inc(s_atlo, 16)
            sync.dma_start(out=wct_hi_sb, in_=wct_hi[:]).then_inc(s_wcthi, 16)
            sync.dma_start(out=wct_lo_sb, in_=wct_lo[:]).then_inc(s_wctlo, 16)
            sync.dma_start(out=vecs_sb, in_=vecs[:]).then_inc(s_vecs, 16)
            sync.dma_start(out=wj_sb, in_=wj[:]).then_inc(s_wj, 16)
            sync.dma_start(out=bvec_sb, in_=bvec[:]).then_inc(s_bvec, 16)
            sync.dma_start(out=xrt_sb, in_=xrt[:]).then_inc(s_xrt, 16)
            for s in range(1, T):
                # slab of z_s ready once its 4 copies are done
                sync.wait_ge(s_cp, 4 * s)
                sync.dma_start(
                    out=zslab[s - 1][:].rearrange("(p it) m -> p it m", p=128),
                    in_=znext[(s - 1) % 2],
                ).then_inc(s_slab, 16)
                # gathered z_s -> ring buffer (WAR transitively safe)
                sync.wait_ge(s_cc, s)
                sync.dma_start(
                    out=zbuf[s % 3],
                    in_=zfull[s - 1][:].rearrange("(p t) m -> p t m", p=128),
                ).then_inc(s_zin, 16)
            sync.wait_ge(s_endout, 1)
            sync.dma_start(out=out[:], in_=out_sb).then_inc(s_outdma, 16)

        @block.gpsimd
        def _(gpsimd: bass.BassEngine):
            gpsimd.memset(ident, 0.0)
            gpsimd.affine_select(
                out=ident,
                in_=ident,
                compare_op=mybir.AluOpType.not_equal,
                fill=1.0,
                base=0,
                pattern=[[-1, OUT]],
                channel_multiplier=1,
            ).then_inc(s_ident, 1)
            for s in range(1, T):
                gpsimd.wait_ge(s_slab, 16 * s)
                gpsimd.collective_compute(
                    "AllGather",
                    mybir.AluOpType.bypass,
                    replica_groups=groups,
                    ins=[zslab[s - 1][:]],
                    outs=[zfull[s - 1][:]],
                ).then_inc(s_cc, 1)

        def chain_mms(tensor, it, zh, hilo):
            for t in range(NJT):
                sl = at_hi_sb[:, t, it * 128 : (it + 1) * 128]
                if hilo:
                    tensor.matmul(
                        pszn[it], lhsT=sl, rhs=zh[:, t, 0:2],
                        start=(t == 0), stop=False,
                    )
                    tensor.matmul(
                        pszn[it], lhsT=sl, rhs=zh[:, t, 2:4],
                        start=False, stop=False,
                    )
                    mm = tensor.matmul(
                        pszn[it],
                        lhsT=at_lo_sb[:, t, it * 128 : (it + 1) * 128],
                        rhs=zh[:, t, 0:2],
                        start=False, stop=(t == NJT - 1),
                    )
                else:
                    mm = tensor.matmul(
                        pszn[it], lhsT=sl, rhs=zh[:, t, 0:2],
                        start=(t == 0), stop=(t == NJT - 1),
                    )
            return mm

        def proj_mms(tensor, j, zh, hilo):
            for t in range(NJT):
                if hilo:
                    tensor.matmul(
                        proj[:, j, :], lhsT=wct_hi_sb[:, t, :], rhs=zh[:, t, 0:2],
                        start=(t == 0), stop=False,
                    )
                    tensor.matmul(
                        proj[:, j, :], lhsT=wct_hi_sb[:, t, :], rhs=zh[:, t, 2:4],
                        start=False, stop=False,
                    )
                    pr = tensor.matmul(
                        proj[:, j, :], lhsT=wct_lo_sb[:, t, :], rhs=zh[:, t, 0:2],
                        start=False, stop=(t == NJT - 1),
                    )
                else:
                    pr = tensor.matmul(
                        proj[:, j, :], lhsT=wct_hi_sb[:, t, :], rhs=zh[:, t, 0:2],
                        start=(t == 0), stop=(t == NJT - 1),
                    )
            return pr

        @block.tensor
        def _(tensor: bass.BassEngine):
            tensor.wait_ge(s_athi, 16)
            tensor.wait_ge(s_atlo, 16)
            tensor.wait_ge(s_wcthi, 16)
            tensor.wait_ge(s_wctlo, 16)
            for s in range(1, T):
                tensor.wait_ge(s_zprep, s)  # z_{s-1} split ready
                if s >= 2:
                    tensor.wait_ge(s_cp, 4 * (s - 1))  # psum[it] drained
                zh = zhl[(s - 1) % 3]
                for it in range(NIT):
                    mm = chain_mms(tensor, it, zh, hilo=(s <= S0))
                    mm.then_inc(s_mm, 1)
                # projections of z_{s-1} while the AllGather flies
                pr = proj_mms(tensor, s - 1, zh, hilo=(s - 1 <= S0))
                pr.then_inc(s_proj, 1)
            # final projection of z_{T-1}
            tensor.wait_ge(s_zprep, T)
            pr = proj_mms(tensor, T - 1, zhl[(T - 1) % 3], hilo=False)
            pr.then_inc(s_proj, 1)
            # endgame: transpose ktilT -> [s, o], then the output matmul
            tensor.wait_ge(s_ktilT, 1)
            tensor.wait_ge(s_ident, 1)
            tensor.transpose(tp_ps, ktilT, ident).then_inc(s_tp, 1)
            tensor.wait_ge(s_ktil2, 1)
            tensor.wait_ge(s_xrt, 16)
            tensor.matmul(out_ps, lhsT=xrt_sb, rhs=ktil, start=True, stop=True).then_inc(
                s_outmm, 1
            )

        def zprep(vector, j):
            """derive zhl[j%3] (bf16 hi/lo) from zbuf[j%3] (fp32)."""
            zb = zbuf[j % 3]
            zh = zhl[j % 3]
            vector.tensor_copy(zh[:, :, 0:2], zb)
            if j <= S0:
                vector.drain()
                vector.tensor_copy(zhi32, zh[:, :, 0:2])
                vector.drain()
                vector.tensor_sub(ztmp, zb, zhi32)
                vector.drain()
                return vector.tensor_copy(zh[:, :, 2:4], ztmp)
            return vector.tensor_copy(zh[0:1, 0, 0:1], zh[0:1, 0, 0:1])

        @block.vector
        def _(vector: bass.BassEngine):
            # z_0 = [v | c]
            vector.wait_ge(s_vecs, 16)
            vector.tensor_copy(zbuf[0][:, :, 0], vecs_sb[:, 0, :])
            vector.tensor_add(csum, vecs_sb[:, 1, :], vecs_sb[:, 2, :])
            vector.drain()
            vector.tensor_add(zbuf[0][:, :, 1], csum, vecs_sb[:, 3, :])
            vector.drain()
            zprep(vector, 0).then_inc(s_zprep, 1)
            for s in range(1, T):
                if s >= 3:
                    # znext[(s-1)%2] was drained by the slab DMA of step s-2
                    vector.wait_ge(s_slab, 16 * (s - 2))
                for it in range(NIT):
                    vector.wait_ge(s_mm, 4 * (s - 1) + it + 1)
                    vector.tensor_copy(
                        znext[(s - 1) % 2][:, it, :], pszn[it]
                    ).then_inc(s_cp, 1)
                # split the gathered z_s once it lands
                vector.wait_ge(s_zin, 16 * s)
                zprep(vector, s).then_inc(s_zprep, 1)
            # endgame: build ktilT = [Ktil^T | const column]
            vector.wait_ge(s_proj, T)
            vector.tensor_copy(ktilT[:, 0:T], proj[:, :, 0])
            vector.wait_ge(s_bvec, 16)
            vector.tensor_add(ktilT[:, 0:1], proj[:, 0, 0:1], bvec_sb[:, 3:4])
            vector.tensor_reduce(
                dsum, proj[:, :, 1], mybir.AxisListType.X, mybir.AluOpType.add
            )
            vector.wait_ge(s_wj, 16)
            vector.tensor_reduce(
                wjsum, wj_sb, mybir.AxisListType.X, mybir.AluOpType.add
            )
            vector.tensor_add(acc1, bvec_sb[:, 0:1], bvec_sb[:, 1:2])
            vector.drain()
            vector.tensor_add(acc2, acc1, bvec_sb[:, 2:3])
            vector.drain()
            vector.tensor_add(acc3, acc2, wjsum)
            vector.drain()
            vector.tensor_add(ktilT[:, T : T + 1], acc3, dsum).then_inc(s_ktilT, 1)
            vector.wait_ge(s_tp, 1)
            vector.tensor_copy(ktil, tp_ps).then_inc(s_ktil2, 1)
            vector.wait_ge(s_outmm, 1)
            vector.tensor_copy(out_sb, out_ps).then_inc(s_endout, 1)

    return nc


_NC_CACHE = None


def _perm_major(vec):
    """(H,) hidden-indexed vector -> [128, NJT] partition-major layout."""
    return np.ascontiguousarray(vec.reshape(128, NJT))


def kernel(**inputs) -> np.ndarray:
    global LAST_RESULT, _NC_CACHE
    import ml_dtypes

    bf = ml_dtypes.bfloat16
    x = np.asarray(inputs["x"], np.float32)
    W_A = np.asarray(inputs["W_A"], np.float32)
    b_A = np.asarray(inputs["b_A"], np.float32)
    W_B = np.asarray(inputs["W_B"], np.float32)
    b_B = np.asarray(inputs["b_B"], np.float32)
    W_bh = np.asarray(inputs["W_bh"], np.float32)
    W_C = np.asarray(inputs["W_C"], np.float32)
    b_C = np.asarray(inputs["b_C"], np.float32)
    W_D = np.asarray(inputs["W_D"], np.float32)
    b_D = np.asarray(inputs["b_D"], np.float32)
    W_J = np.asarray(inputs["W_J"], np.float32)
    b_J = np.asarray(inputs["b_J"], np.float32)

    if _NC_CACHE is None:
        _NC_CACHE = _build()
    nc = _NC_CACHE

    # x reversed/truncated + ones row
    xr = x[:, ::-1, 0][:, :T]  # Xr[b, s] = x[b, S-1-s]
    xrt = np.concatenate(
        [np.ascontiguousarray(xr.T), np.ones((1, B), np.float32)], axis=0
    )

    # W_A^T column slab per core, rows partition-major, columns ordered so
    # that slab row r = p*NIT + it of the step output corresponds to the
    # matmul's (it, p) psum element: column slot c = it*128 + p holds the
    # original column 512k + (c % 128)*NIT + c // 128.
    WAT = W_A.T  # [j, i]
    c = np.arange(HSH)
    colperm = (c % 128) * NIT + c // 128  # original column for slot c
    vecs = np.ascontiguousarray(
        np.stack(
            [_perm_major(W_B[:, 0]), _perm_major(b_A), _perm_major(b_B),
             _perm_major(W_bh)],
            axis=1,
        )
    )  # [128, 4, NJT]
    bvec = np.ascontiguousarray(
        np.stack([b_C, b_D, b_J, W_D[:, 0]], axis=1)
    )  # [OUT, 4]
    wct = W_C.T.reshape(128, NJT, OUT)
    wct_hi = wct.astype(bf)
    wct_lo = (wct - wct_hi.astype(np.float32)).astype(bf)
    common = dict(
        wct_hi=np.ascontiguousarray(wct_hi),
        wct_lo=np.ascontiguousarray(wct_lo),
        vecs=vecs,
        wj=W_J,
        bvec=bvec,
        xrt=xrt,
    )
    in_maps = []
    for k in range(NCORES):
        slab = WAT[:, k * HSH + colperm].reshape(128, NJT, HSH)
        hi = slab.astype(bf)
        lo = (slab - hi.astype(np.float32)).astype(bf)
        in_maps.append(
            {"at_hi": np.ascontiguousarray(hi), "at_lo": np.ascontiguousarray(lo),
             **common}
        )

    import os

    trace = bool(os.environ.get("BASS_TRACE"))
    LAST_RESULT = run_bass_kernel_spmd(
        nc, in_maps, list(range(NCORES)), trace=trace
    )
    return np.asarray(LAST_RESULT.results[0]["out"], np.float32)
